# revision 1
# baseline (speedup 1.0000x reference)
"""Trainium2 Bass kernel for a cross-attention decoder block.

Problem (hardcoded shapes): B=2, LQ=LK=2048, D=512, H=8 heads (hd=64), DFF=2048.

    q = x @ Wq; k = enc @ Wk; v = enc @ Wv            (per batch)
    attn = softmax(q k^T / sqrt(hd)); o = attn v
    out1 = LayerNorm(o + x)
    y = LayerNorm(relu(out1 @ W1 + b1) @ W2 + b2 + out1)

Sharding: row-parallel over the 4096 flattened query rows; 8 cores x 512 rows.
Cores 0-3 take batch 0, cores 4-7 batch 1 (each core's rows stay inside one
batch). Every core receives its batch's full encoder_x and all weights and
computes K/V for its batch locally (replicated within the 4-core batch group)
-- no collectives at all.

Matmuls run in float32r (TF32-like, ~1.5e-4 rel err per matmul, 4x faster
than fp32 on the PE). Softmax skips max-subtraction: scores with these
Xavier-scale weights are O(10), far from exp overflow.
"""

import sys

sys.path.insert(0, "/opt/trn_rl_repo")

import numpy as np

import concourse.bacc as bacc
import concourse.bass as bass
import concourse.mybir as mybir
from concourse import masks, tile
from concourse.bass_utils import run_bass_kernel_spmd

F32 = mybir.dt.float32
F32R = mybir.dt.float32r

B, LQ, LK, D, H, DFF = 2, 2048, 2048, 512, 8, 2048
HD = D // H  # 64
N_CORES = 8
ROWS = B * LQ // N_CORES  # 512 query rows per core
RT = ROWS // 128  # 4 row tiles
DT = D // 128  # 4 d tiles
LT = LK // 128  # 16 lk tiles
FT = DFF // 128  # 16 dff tiles
EPS = 1e-5


def build_program() -> bass.Bass:
    nc = bacc.Bacc(None, target_bir_lowering=False, debug=False)

    x_d = nc.dram_tensor("x", [ROWS, D], F32, kind="ExternalInput")
    enc_d = nc.dram_tensor("enc", [LK, D], F32, kind="ExternalInput")
    wq_d = nc.dram_tensor("wq", [D, D], F32, kind="ExternalInput")
    wk_d = nc.dram_tensor("wk", [D, D], F32, kind="ExternalInput")
    wv_d = nc.dram_tensor("wv", [D, D], F32, kind="ExternalInput")
    w1_d = nc.dram_tensor("w1", [D, DFF], F32, kind="ExternalInput")
    w2_d = nc.dram_tensor("w2", [DFF, D], F32, kind="ExternalInput")
    b1_d = nc.dram_tensor("b1", [DFF], F32, kind="ExternalInput")
    b2_d = nc.dram_tensor("b2", [D], F32, kind="ExternalInput")
    g1_d = nc.dram_tensor("g1", [D], F32, kind="ExternalInput")
    be1_d = nc.dram_tensor("be1", [D], F32, kind="ExternalInput")
    g2_d = nc.dram_tensor("g2", [D], F32, kind="ExternalInput")
    be2_d = nc.dram_tensor("be2", [D], F32, kind="ExternalInput")
    y_d = nc.dram_tensor("y", [ROWS, D], F32, kind="ExternalOutput")

    from contextlib import ExitStack

    with ExitStack() as ctx:
        tc = ctx.enter_context(tile.TileContext(nc))
        cpool = ctx.enter_context(tc.tile_pool(name="const", bufs=1))
        stpool = ctx.enter_context(tc.tile_pool(name="stage", bufs=4))
        # f32r [128,512]: wq/wk/wv ktiles, xT, out1T
        wpool = ctx.enter_context(tc.tile_pool(name="wproj", bufs=16))
        xpool = ctx.enter_context(tc.tile_pool(name="xsb", bufs=RT))
        # f32r [128,2048]: encT then w1r
        bigpool = ctx.enter_context(tc.tile_pool(name="big8k", bufs=DT))
        # f32r [128,2048]: KT; then w2r [128,4,512]x4
        ktpool = ctx.enter_context(tc.tile_pool(name="ktp", bufs=DT))
        vpool = ctx.enter_context(tc.tile_pool(name="vaug", bufs=1))  # vaug then h1T
        qpool = ctx.enter_context(tc.tile_pool(name="qt", bufs=DT))
        epool = ctx.enter_context(tc.tile_pool(name="expt", bufs=3))  # f32r [128,1024]
        opool = ctx.enter_context(tc.tile_pool(name="ohead", bufs=2))
        oypool = ctx.enter_context(tc.tile_pool(name="oy", bufs=RT))  # o_sb then y
        o1pool = ctx.enter_context(tc.tile_pool(name="out1", bufs=RT))
        spool = ctx.enter_context(tc.tile_pool(name="stat", bufs=10))
        pbig = ctx.enter_context(tc.tile_pool(name="pbig", bufs=2, space="PSUM"))
        pacc = ctx.enter_context(tc.tile_pool(name="pacc", bufs=4, space="PSUM"))
        ptr = pacc  # transposes share the accumulator bank slots
        if True:
            # ---- constants ----
            ident = cpool.tile([128, 128], F32)
            masks.make_identity(nc, ident[:])

            def bcast_row(dram_vec, name):
                row = cpool.tile([1, D], F32, name=f"{name}_row")
                nc.sync.dma_start(row[:], dram_vec[None, :])
                full = cpool.tile([128, D], F32, name=f"{name}_bc")
                nc.gpsimd.partition_broadcast(full[:], row[:])
                return full

            eps_col = cpool.tile([128, 1], F32)
            nc.gpsimd.memset(eps_col[:], EPS)

            # ---- stage A: interleaved pipeline ----
            # DMA order: wk, enc[0], wv, enc[1], x, enc[2], wq, enc[3], w1.
            # Each enc chunk: transpose -> encT, then KT chunk + V tiles, so PE
            # work starts as soon as the first chunk lands.
            def load_w_512(dram, name):
                tiles = []
                for kt in range(DT):
                    s = stpool.tile([128, D], F32, name=f"{name}st{kt}", tag="stage")
                    nc.sync.dma_start(s[:], dram[kt * 128 : (kt + 1) * 128, :])
                    t = wpool.tile([128, D], F32R, name=f"{name}r{kt}", tag="w512r")
                    nc.gpsimd.tensor_copy(t[:], s[:])
                    tiles.append(t)
                return tiles

            wkr = []

            # V store: per (lk-tile, head-pair) slot [V_even(64) | 1 | V_odd(64) | 1]
            # -> per-head attnV lhsT is a contiguous 65-col window; out row 64
            # is the softmax denominator.
            PSLOT = 130
            TSLOT = 4 * PSLOT  # 520 per lk-tile
            vaug = vpool.tile([128, LT * TSLOT], F32R, tag="vh")
            ones128 = cpool.tile([128, 128], F32)
            nc.gpsimd.memset(ones128[:], 1.0)
            nc.gpsimd.tensor_copy(
                bass.AP(
                    tensor=vaug.tensor,
                    offset=vaug.offset + 64,
                    ap=[list(vaug.ap[0]), [TSLOT, LT], [65, 8]],
                ),
                ones128[:].rearrange("p (a b) -> p a b", b=8),
            )

            encT = [bigpool.tile([128, LK], F32R, name=f"encT{d}", tag="big8k") for d in range(DT)]
            KT = [ktpool.tile([128, LK], F32R, name=f"KT{ct}", tag="ktw2") for ct in range(DT)]
            x_sb = []
            xT = []
            wvr = []
            wqr = []

            def emit_x_and_xT():
                for rt in range(RT):
                    xt_ = xpool.tile([128, D], F32, name=f"x{rt}", tag="x")
                    nc.sync.dma_start(xt_[:], x_d[rt * 128 : (rt + 1) * 128, :])
                    x_sb.append(xt_)
                for dt_ in range(DT):
                    pt = pacc.tile([128, ROWS], F32, name=f"pxT{dt_}", tag="pacc")
                    for rt in range(RT):
                        nc.tensor.matmul(
                            pt[:, rt * 128 : (rt + 1) * 128],
                            x_sb[rt][:, dt_ * 128 : (dt_ + 1) * 128],
                            ident[:],
                            is_transpose=True,
                            start=(rt == 0),
                            stop=(rt == RT - 1),
                        )
                    t = wpool.tile([128, ROWS], F32R, name=f"xT{dt_}", tag="w512r")
                    nc.scalar.copy(t[:], pt[:])
                    xT.append(t)

            for c4 in range(LT // 4):
                stg = []
                for j in range(4):
                    lkr = c4 * 4 + j
                    s = stpool.tile([128, D], F32, name=f"encst{lkr}", tag="stage")
                    nc.sync.dma_start(s[:], enc_d[lkr * 128 : (lkr + 1) * 128, :])
                    stg.append(s)
                # interleave the other loads between enc chunks
                if c4 == 0:
                    wkr.extend(load_w_512(wk_d, "wk"))
                elif c4 == 1:
                    wvr.extend(load_w_512(wv_d, "wv"))
                elif c4 == 2:
                    emit_x_and_xT()
                elif c4 == 3:
                    wqr.extend(load_w_512(wq_d, "wq"))
                for dt_ in range(DT):
                    pt = pacc.tile([128, 512], F32, name=f"peT{c4}_{dt_}", tag="pacc")
                    for j in range(4):
                        nc.tensor.matmul(
                            pt[:, j * 128 : (j + 1) * 128],
                            stg[j][:, dt_ * 128 : (dt_ + 1) * 128],
                            ident[:],
                            is_transpose=True,
                            start=(j == 0),
                            stop=(j == 3),
                        )
                    nc.scalar.copy(encT[dt_][:, c4 * 512 : (c4 + 1) * 512], pt[:])
                # KT chunk c4 for all 4 output tiles
                for ct in range(DT):
                    ps = pbig.tile([128, 512], F32, name=f"pk{ct}_{c4}", tag="pbig")
                    for kt in range(DT):
                        nc.tensor.matmul(
                            ps[:],
                            wkr[kt][:, ct * 128 : (ct + 1) * 128],
                            encT[kt][:, c4 * 512 : (c4 + 1) * 512],
                            start=(kt == 0),
                            stop=(kt == DT - 1),
                        )
                    nc.vector.tensor_copy(KT[ct][:, c4 * 512 : (c4 + 1) * 512], ps[:])
                # V tiles of this chunk (needs wvr -> only from chunk 1 on)
                if c4 >= 1:
                    lo = 4 if c4 == 1 else c4 * 4
                    hi = c4 * 4 + 4
                    if c4 == 1:
                        lo = 0
                    for t in range(lo, hi):
                        ps = pbig.tile([128, D], F32, name=f"pv{t}", tag="pbig")
                        for kt in range(DT):
                            nc.tensor.matmul(
                                ps[:],
                                encT[kt][:, t * 128 : (t + 1) * 128],
                                wvr[kt][:],
                                start=(kt == 0),
                                stop=(kt == DT - 1),
                            )
                        nc.vector.tensor_copy(
                            bass.AP(
                                tensor=vaug.tensor,
                                offset=vaug.offset + t * TSLOT,
                                ap=[list(vaug.ap[0]), [PSLOT, 4], [65, 2], [1, 64]],
                            ),
                            ps[:].rearrange("p (pr s c) -> p pr s c", pr=4, c=64),
                        )

            # ---- qT = Wq.T @ xT -> [128, ROWS] x DT (f32r) ----
            qT = []
            for ct in range(DT):
                ps = pbig.tile([128, ROWS], F32, name=f"pq{ct}", tag="pbig")
                for kt in range(DT):
                    nc.tensor.matmul(
                        ps[:],
                        wqr[kt][:, ct * 128 : (ct + 1) * 128],
                        xT[kt][:],
                        start=(kt == 0),
                        stop=(kt == DT - 1),
                    )
                t = qpool.tile([128, ROWS], F32R, name=f"qT{ct}", tag="qT")
                nc.scalar.copy(t[:], ps[:])
                qT.append(t)

            bc_g1 = bcast_row(g1_d, "g1")
            bc_be1 = bcast_row(be1_d, "be1")
            bc_g2 = bcast_row(g2_d, "g2")
            bc_be2 = bcast_row(be2_d, "be2")
            bc_b2 = bcast_row(b2_d, "b2")
            # b1 as per-partition scalars in h1T layout: [128, FT]
            b1col = cpool.tile([128, FT], F32)
            nc.sync.dma_start(b1col[:], b1_d.rearrange("(t p) -> p t", p=128))

            def vaug_lhsT(h, t):
                # contiguous [128, 65]: head h's V columns in tile t + ones col
                off = t * TSLOT + (h // 2) * PSLOT + (h % 2) * 65
                return bass.AP(
                    tensor=vaug.tensor,
                    offset=vaug.offset + off,
                    ap=[list(vaug.ap[0]), [1, 65]],
                )

            # ---- prefetch W1 (f32r) into the encT slots ----
            w1r = []
            for kt in range(DT):
                t = bigpool.tile([128, DFF], F32R, name=f"w1r{kt}", tag="big8k")
                for c in range(DFF // 512):
                    s = stpool.tile([128, 512], F32, name=f"w1st{kt}_{c}", tag="stage")
                    nc.sync.dma_start(
                        s[:], w1_d[kt * 128 : (kt + 1) * 128, c * 512 : (c + 1) * 512]
                    )
                    nc.gpsimd.tensor_copy(t[:, c * 512 : (c + 1) * 512], s[:])
                w1r.append(t)

            # ---- attention: head pairs, scoresT chunks of 2 lk-tiles ----
            o_sb = [oypool.tile([128, D], F32, name=f"osb{rt}", tag="oy") for rt in range(RT)]
            w2r = []
            CHUNKS = [(0, 2), (2, 2), (4, 2), (6, 2), (8, 2), (10, 2), (12, 2), (14, 2)]
            for h in range(H):
                pr = h // 2
                off = 64 * (h % 2)
                KTh = KT[pr]
                acc = pacc.tile([65, ROWS], F32, name=f"acc{h}", tag="pacc")
                for t0, n in CHUNKS:
                    sc = pbig.tile([128, 512 * n], F32, name=f"sc{h}_{t0}", tag="pbig")
                    for j in range(n):
                        t = t0 + j
                        nc.tensor.matmul(
                            sc[:, j * 512 : (j + 1) * 512],
                            KTh[off : off + 64, t * 128 : (t + 1) * 128],
                            qT[pr][off : off + 64, :],
                            start=True,
                            stop=True,
                            tile_position=(off, 0),
                        )
                    e = epool.tile([128, 512 * n], F32R, name=f"e{h}_{t0}", tag="e")
                    nc.scalar.activation(
                        e[:], sc[:], mybir.ActivationFunctionType.Exp, scale=0.125
                    )
                    for j in range(n):
                        t = t0 + j
                        nc.tensor.matmul(
                            acc[:],
                            vaug_lhsT(h, t),
                            e[:, j * 512 : (j + 1) * 512],
                            start=(t == 0),
                            stop=(t == LT - 1),
                        )
                # normalize + transpose into o_sb
                oh = opool.tile([65, ROWS], F32, name=f"oh{h}", tag="oh")
                nc.vector.tensor_copy(oh[:], acc[:])
                for rt in range(RT):
                    pt = pacc.tile([128, 65], F32, name=f"pot{h}_{rt}", tag="pacc")
                    nc.tensor.matmul(
                        pt[:],
                        oh[:, rt * 128 : (rt + 1) * 128],
                        ident[0:65, 0:65],
                        is_transpose=True,
                        start=True,
                        stop=True,
                    )
                    rec = spool.tile([128, 1], F32, name=f"rec{h}_{rt}", tag="stat")
                    nc.vector.reciprocal(rec[:], pt[:, 64:65])
                    nc.vector.tensor_scalar(
                        o_sb[rt][:, h * 64 : (h + 1) * 64],
                        pt[:, 0:64],
                        rec[:, 0:1],
                        None,
                        mybir.AluOpType.mult,
                    )

                if h % 2 == 1:
                    # W2 chunk pr reuses KT[pr]'s slot (dead after this head's scores)
                    w2t = ktpool.tile([128, 4, D], F32R, name=f"w2r{pr}", tag="ktw2")
                    for j in range(4):
                        ft = pr * 4 + j
                        s = stpool.tile([128, D], F32, name=f"w2st{ft}", tag="stage")
                        nc.sync.dma_start(s[:], w2_d[ft * 128 : (ft + 1) * 128, :])
                        nc.gpsimd.tensor_copy(w2t[:, j, :], s[:])
                    w2r.append(w2t)

            # ---- layernorm helper (in-place on `t`, writes normalized out) ----
            def layer_norm(t, gain_bc, bias_bc, name, apply_gb=True):
                bn6 = spool.tile([128, 6], F32, name=f"bn6{name}", tag="stat")
                nc.vector.bn_stats(bn6[:], t[:])
                mv = spool.tile([128, 2], F32, name=f"mv{name}", tag="stat")
                nc.vector.bn_aggr(mv[:], bn6[:])
                std = spool.tile([128, 1], F32, name=f"std{name}", tag="stat")
                nc.scalar.activation(
                    std[:],
                    mv[:, 1:2],
                    mybir.ActivationFunctionType.Sqrt,
                    bias=eps_col[:, 0:1],
                )
                rstd = spool.tile([128, 1], F32, name=f"rstd{name}", tag="stat")
                nc.vector.reciprocal(rstd[:], std[:])
                nc.vector.tensor_scalar(
                    t[:],
                    t[:],
                    mv[:, 0:1],
                    rstd[:, 0:1],
                    mybir.AluOpType.subtract,
                    mybir.AluOpType.mult,
                )
                if apply_gb:
                    nc.vector.tensor_tensor(t[:], t[:], gain_bc[:], mybir.AluOpType.mult)
                    nc.vector.tensor_tensor(t[:], t[:], bias_bc[:], mybir.AluOpType.add)

            # ---- residual + LN1 -> out1; out1T ----
            out1 = []
            for rt in range(RT):
                t = o1pool.tile([128, D], F32, name=f"out1_{rt}", tag="out1")
                nc.vector.tensor_tensor(t[:], x_sb[rt][:], o_sb[rt][:], mybir.AluOpType.add)
                # g1/b1 are folded into W1/b1 host-side; o1T takes the pre-gain
                # normalized value, g/b applied afterwards (for the LN2 residual)
                layer_norm(t, bc_g1, bc_be1, f"ln1_{rt}", apply_gb=False)
                out1.append(t)
            o1T = []
            for dt_ in range(DT):
                pt = ptr.tile([128, ROWS], F32, name=f"po1T{dt_}", tag="pacc")
                for rt in range(RT):
                    nc.tensor.matmul(
                        pt[:, rt * 128 : (rt + 1) * 128],
                        out1[rt][:, dt_ * 128 : (dt_ + 1) * 128],
                        ident[:],
                        is_transpose=True,
                        start=(rt == 0),
                        stop=(rt == RT - 1),
                    )
                t = wpool.tile([128, ROWS], F32R, name=f"o1T{dt_}", tag="w512r")
                nc.scalar.copy(t[:], pt[:])
                o1T.append(t)

            for rt in range(RT):
                nc.vector.tensor_tensor(
                    out1[rt][:], out1[rt][:], bc_g1[:], mybir.AluOpType.mult
                )
                nc.vector.tensor_tensor(
                    out1[rt][:], out1[rt][:], bc_be1[:], mybir.AluOpType.add
                )

            # ---- FFN1 (h1T = relu(W1.T @ out1T + b1)) with FFN2 chains for
            # rows 0-1 accumulating right behind it on the pacc slots ----
            h1T = vpool.tile([128, FT, ROWS], F32R, name="h1T", tag="vh")
            f2ps = [
                pacc.tile([128, D], F32, name=f"pf2{rt}", tag="pacc") for rt in range(2)
            ]

            def emit_ffn2_mm(ps, rt, ft):
                nc.tensor.matmul(
                    ps[:],
                    h1T[:, ft, rt * 128 : (rt + 1) * 128],
                    w2r[ft // 4][:, ft % 4, :],
                    start=(ft == 0),
                    stop=(ft == FT - 1),
                )

            def emit_ffn2_tail(ps, rt):
                yt = oypool.tile([128, D], F32, name=f"y{rt}", tag="oy")
                nc.vector.tensor_tensor(yt[:], ps[:], bc_b2[:], mybir.AluOpType.add)
                nc.vector.tensor_tensor(yt[:], yt[:], out1[rt][:], mybir.AluOpType.add)
                layer_norm(yt, bc_g2, bc_be2, f"ln2_{rt}")
                nc.sync.dma_start(y_d[rt * 128 : (rt + 1) * 128, :], yt[:])

            for c4 in range(FT // 4):
                for j in range(4):
                    ct = c4 * 4 + j
                    ps = pbig.tile([128, ROWS], F32, name=f"ph1{ct}", tag="pbig")
                    for kt in range(DT):
                        nc.tensor.matmul(
                            ps[:],
                            w1r[kt][:, ct * 128 : (ct + 1) * 128],
                            o1T[kt][:],
                            start=(kt == 0),
                            stop=(kt == DT - 1),
                        )
                    nc.vector.tensor_scalar(
                        h1T[:, ct, :],
                        ps[:],
                        b1col[:, ct : ct + 1],
                        0.0,
                        mybir.AluOpType.add,
                        mybir.AluOpType.max,
                    )
                for rt in range(2):
                    for j in range(4):
                        emit_ffn2_mm(f2ps[rt], rt, c4 * 4 + j)
            for rt in range(2):
                emit_ffn2_tail(f2ps[rt], rt)
            # rows 2-3 reuse the freed pacc slots
            for rt in range(2, RT):
                ps = pacc.tile([128, D], F32, name=f"pf2{rt}", tag="pacc")
                for ft in range(FT):
                    emit_ffn2_mm(ps, rt, ft)
                emit_ffn2_tail(ps, rt)

    nc.compile()
    return nc


_CACHED_NC = None


def _get_nc():
    global _CACHED_NC
    if _CACHED_NC is None:
        _CACHED_NC = build_program()
    return _CACHED_NC


def kernel(**inputs) -> np.ndarray:
    x = np.ascontiguousarray(np.asarray(inputs["inputs"], dtype=np.float32))
    enc = np.ascontiguousarray(np.asarray(inputs["encoder_x"], dtype=np.float32))
    b, lq, d = x.shape
    assert (b, lq, d) == (B, LQ, D)
    assert int(np.asarray(inputs["n_heads"])) == H

    g1 = np.asarray(inputs["ln1_g"], np.float64)
    be1 = np.asarray(inputs["ln1_b"], np.float64)
    w1_raw = np.asarray(inputs["W1"], np.float64)
    w1_eff = (g1[:, None] * w1_raw).astype(np.float32)
    b1_eff = (np.asarray(inputs["b1"], np.float64) + be1 @ w1_raw).astype(np.float32)
    shared = {
        "wq": np.ascontiguousarray(np.asarray(inputs["Wq"], np.float32)),
        "wk": np.ascontiguousarray(np.asarray(inputs["Wk"], np.float32)),
        "wv": np.ascontiguousarray(np.asarray(inputs["Wv"], np.float32)),
        "w1": np.ascontiguousarray(w1_eff),
        "w2": np.ascontiguousarray(np.asarray(inputs["W2"], np.float32)),
        "b1": np.ascontiguousarray(b1_eff),
        "b2": np.ascontiguousarray(np.asarray(inputs["b2"], np.float32)),
        "g1": np.ascontiguousarray(np.asarray(inputs["ln1_g"], np.float32)),
        "be1": np.ascontiguousarray(np.asarray(inputs["ln1_b"], np.float32)),
        "g2": np.ascontiguousarray(np.asarray(inputs["ln2_g"], np.float32)),
        "be2": np.ascontiguousarray(np.asarray(inputs["ln2_b"], np.float32)),
    }
    xf = x.reshape(B * LQ, D)
    in_maps = []
    for c in range(N_CORES):
        m = dict(shared)
        m["x"] = np.ascontiguousarray(xf[c * ROWS : (c + 1) * ROWS])
        m["enc"] = np.ascontiguousarray(enc[c // (N_CORES // B)])
        in_maps.append(m)

    nc = _get_nc()
    res = run_bass_kernel_spmd(nc, in_maps, core_ids=list(range(N_CORES)))
    out = np.concatenate([res.results[c]["y"] for c in range(N_CORES)], axis=0)
    return out.reshape(B, LQ, D).astype(np.float32)



# revision 10
# speedup vs baseline: 1.2882x; 1.2882x over previous
"""Trainium2 Bass kernel for a cross-attention decoder block.

Shapes (hardcoded): B=2, LQ=LK=2048, D=512, H=8 heads (hd=64), DFF=2048.

    q = x @ Wq; k = enc @ Wk; v = enc @ Wv            (per batch)
    attn = softmax(q k^T / sqrt(hd)); o = attn v
    out1 = LayerNorm(o + x)
    y = LayerNorm(relu(out1 @ W1 + b1) @ W2 + b2 + out1)

Sharding: row-parallel over the 4096 flattened query rows; 8 cores x 512 rows.
Cores 0-3 take batch 0, cores 4-7 batch 1. Each core computes its batch's full
K/V locally (replicated within the 4-core group) -- no collectives.

Numerics (validated vs the reference in fp emulation, rel ~7e-3):
  - host pre-quantizes operands: x^T/enc^T/Wq/Wk/Wv in fp8e4m3 (pow-2 scales),
    W1/W2/x in bf16; all transposes are done on the host for free.
  - q/k/v projections run in fp8 DoubleRow mode (2 k-tiles per matmul, 0.5
    cycles/row = 4x the f32r rate).
  - scores run in fp8 DoubleRow with a ZERO second k-tile (a zero tail inside
    the KT/qT tensors reached by a step-sliced AP keeps it in-bounds):
    contraction is only hd=64 but the column cost still halves.
  - softmax exp: 6 of 8 lk-chunks per head on ACT (Exp activation, fp8 out,
    offset e^-3 so max e' ~ 126 < 448), 1 chunk on DVE + 1 on Pool via an
    int16-bitcast-bf16 exp trick: bits = round(23.083*qk + bias) as int16 ==
    bf16(e^(qk/8-3) * (1 +- 1.8%)); the 23.083 slope is folded into the KT
    fp8 quantization of those lk columns.
  - attn@V: fp8 DoubleRow for ACT chunks, bf16 for hack chunks, accumulated
    into one PSUM tile; a 16.0 "ones" column yields the softmax denominator.
  - FFN stays bf16 (fp8 FFN measured at 1.5e-2 error -- too close to the
    2e-2 gate).
"""

import sys

sys.path.insert(0, "/opt/trn_rl_repo")

from contextlib import ExitStack

import numpy as np
import ml_dtypes

import concourse.bacc as bacc
import concourse.bass as bass
import concourse.mybir as mybir
from concourse import masks, tile
from concourse.bass_utils import run_bass_kernel_spmd

F32 = mybir.dt.float32
BF16 = mybir.dt.bfloat16
F8 = mybir.dt.float8e4
I16 = mybir.dt.int16
F8NP = ml_dtypes.float8_e4m3fn
BF16NP = ml_dtypes.bfloat16

B, LQ, LK, D, H, DFF = 2, 2048, 2048, 512, 8, 2048
HD = D // H  # 64
N_CORES = 8
ROWS = B * LQ // N_CORES  # 512 query rows per core
RT = ROWS // 128  # 4 row tiles
DT = D // 128  # 4 d tiles
LT = LK // 128  # 16 lk tiles
FT = DFF // 128  # 16 dff tiles
NCH = LT // 2  # 8 exp chunks per head (2 lk tiles each)
EPS = 1e-5
LN2E = float(np.log(2.0))

ACT_CHUNKS = 6  # chunks 0-5 on ACT (fp8 e), chunk 6 on DVE, 7 on Pool (hack)
EOFF = 3.0  # e' = exp(s - EOFF)
HACK_SCALE = 0.125 * 128.0 / LN2E  # 23.083: qk -> bf16-bits slope
HACK_BIAS = 16256.0 - 7.0 - EOFF * 128.0 / LN2E  # folds e^-EOFF into the bits
KTW = (LT + 1) * 128  # KT slab width incl. the zero k-tile tail

DoubleRow = mybir.MatmulPerfMode.DoubleRow
Alu = mybir.AluOpType


def build_program(apply_g2b2: bool, add_b2: bool) -> bass.Bass:
    import os
    PHASE = int(os.environ.get("K_PHASE", "5"))
    KATT = int(os.environ.get("K_ATT", "4"))  # 1=sc 2=+exp 3=+attnV 4=+o-asm
    nc = bacc.Bacc(None, target_bir_lowering=False, debug=False)

    xt8_d = nc.dram_tensor("xt8", [128, DT * ROWS], F8, kind="ExternalInput")
    xb_d = nc.dram_tensor("xb", [128, RT * D], BF16, kind="ExternalInput")
    enct8_d = nc.dram_tensor("enct8", [128, DT * LK], F8, kind="ExternalInput")
    wq8_d = nc.dram_tensor("wq8", [128, DT * D], F8, kind="ExternalInput")
    wk8_d = nc.dram_tensor("wk8", [128, DT * D], F8, kind="ExternalInput")
    wv8_d = nc.dram_tensor("wv8", [128, DT * D], F8, kind="ExternalInput")
    w1b_d = nc.dram_tensor("w1b", [128, DT * DFF], BF16, kind="ExternalInput")
    w2b_d = nc.dram_tensor("w2b", [128, FT * D], BF16, kind="ExternalInput")
    b1c_d = nc.dram_tensor("b1c", [128, FT], F32, kind="ExternalInput")
    g2_d = nc.dram_tensor("g2", [D], F32, kind="ExternalInput")
    be2_d = nc.dram_tensor("be2", [D], F32, kind="ExternalInput")
    b2_d = nc.dram_tensor("b2", [D], F32, kind="ExternalInput")
    y_d = nc.dram_tensor("y", [128, RT * D], F32, kind="ExternalOutput")

    with ExitStack() as ctx:
        tc = ctx.enter_context(tile.TileContext(nc))
        cpool = ctx.enter_context(tc.tile_pool(name="const", bufs=1))
        wpool = ctx.enter_context(tc.tile_pool(name="w8", bufs=4))
        encpool = ctx.enter_context(tc.tile_pool(name="enc8", bufs=1))
        w1pool = ctx.enter_context(tc.tile_pool(name="w1b", bufs=1))
        w2pool = ctx.enter_context(tc.tile_pool(name="w2b", bufs=1))
        xbpool = ctx.enter_context(tc.tile_pool(name="xb", bufs=1))
        qtpool = ctx.enter_context(tc.tile_pool(name="qt8", bufs=1))
        ktpool = ctx.enter_context(tc.tile_pool(name="kt8", bufs=4))
        vpool = ctx.enter_context(tc.tile_pool(name="v8", bufs=1))
        vbpool = ctx.enter_context(tc.tile_pool(name="vb", bufs=1))
        e8pool = ctx.enter_context(tc.tile_pool(name="e8", bufs=2))
        ebpool = ctx.enter_context(tc.tile_pool(name="ebb", bufs=2))
        o1pool = ctx.enter_context(tc.tile_pool(name="o1", bufs=1))
        ob1pool = ctx.enter_context(tc.tile_pool(name="out1b", bufs=1))
        o1tpool = ctx.enter_context(tc.tile_pool(name="o1t", bufs=1))
        h1pool = ctx.enter_context(tc.tile_pool(name="h1t", bufs=1))
        ypool = ctx.enter_context(tc.tile_pool(name="y", bufs=4))
        spool = ctx.enter_context(tc.tile_pool(name="stat", bufs=12))
        # PSUM: pA = 2 slots x 2 banks (warmup/proj/sc/transpose/ffn1),
        # pB + pC = 2 slots x 1 bank each (attnV accums, then FFN2 accums).
        pA = ctx.enter_context(tc.tile_pool(name="pA", bufs=2, space="PSUM"))
        pB = ctx.enter_context(tc.tile_pool(name="pB", bufs=2, space="PSUM"))
        pC = ctx.enter_context(tc.tile_pool(name="pC", bufs=2, space="PSUM"))

        # ---- constants ----
        ident = cpool.tile([128, 128], F32)
        masks.make_identity(nc, ident[:])
        identb = cpool.tile([128, 128], BF16)
        nc.vector.tensor_copy(identb[:], ident[:])
        eps_col = cpool.tile([128, 1], F32)
        nc.gpsimd.memset(eps_col[:], EPS)
        moff_col = cpool.tile([128, 1], F32)
        nc.gpsimd.memset(moff_col[:], -EOFF)

        # ---- PE warmup: keep the PE busy through its p-state ramp (~3us)
        # while the first DMAs land ----
        wsrc = cpool.tile([128, 128], BF16)
        nc.gpsimd.memset(wsrc[:], 0.0)
        for i in range(28):
            wp = pA.tile([128, 128], BF16, name=f"warm{i}", tag="pA")
            nc.tensor.matmul(
                wp[:], wsrc[:], identb[:], is_transpose=True, start=True, stop=True
            )

        # ---- input loads (first-needed first) ----
        def load(pool_, name, dram, cols, dt_):
            t = pool_.tile([128, cols], dt_, name=name, tag=name)
            nc.sync.dma_start(t[:], dram[:, :])
            return t

        xt8 = load(wpool, "xt8", xt8_d, DT * ROWS, F8)
        wq8 = load(wpool, "wq8", wq8_d, DT * D, F8)
        wk8 = load(wpool, "wk8", wk8_d, DT * D, F8)
        enct8 = load(encpool, "enct8", enct8_d, DT * LK, F8)
        wv8 = load(wpool, "wv8", wv8_d, DT * D, F8)
        xb = load(xbpool, "xb", xb_d, RT * D, BF16)
        b1c = load(cpool, "b1c", b1c_d, FT, F32)
        w1b = load(w1pool, "w1b", w1b_d, DT * DFF, BF16)
        w2b = load(w2pool, "w2b", w2b_d, FT * D, BF16)

        xt8v = xt8[:].rearrange("p (n w) -> p n w", w=ROWS)
        wq8v = wq8[:].rearrange("p (n w) -> p n w", w=D)
        wk8v = wk8[:].rearrange("p (n w) -> p n w", w=D)
        wv8v = wv8[:].rearrange("p (n w) -> p n w", w=D)
        enct8v = enct8[:].rearrange("p (n w) -> p n w", w=LK)
        xbv = xb[:].rearrange("p (r d) -> p r d", d=D)

        # ---- qT projection (fp8 DR): qT8[d-slab, q]; zero tail for scores ----
        qt8 = qtpool.tile([128, DT * ROWS + ROWS], F8, name="qt8", tag="qt8")
        nc.gpsimd.memset(qt8[:, DT * ROWS :], 0.0)
        qt8v = qt8[:].rearrange("p (n w) -> p n w", w=ROWS)
        for s in range(DT):
            pq = pA.tile([128, ROWS], F32, name=f"pq{s}", tag="pA")
            for j in range(0, DT, 2):
                nc.tensor.matmul(
                    pq[:],
                    wq8v[:, j : j + 2, s * 128 : (s + 1) * 128],
                    xt8v[:, j : j + 2, :],
                    start=(j == 0),
                    stop=(j == DT - 2),
                    perf_mode=DoubleRow,
                )
            # q = 2^-9 psum; qT8 = fp8(q * 2^3)
            nc.scalar.mul(qt8v[:, s, :], pq[:], 2.0**-6)

        # ---- KT projection (fp8 DR), zero k-tile tail per slab ----
        kt8 = [
            ktpool.tile([128, KTW], F8, name=f"kt8_{s}", tag="kt8") for s in range(DT)
        ]
        for s in range(DT):
            nc.gpsimd.memset(kt8[s][:, LT * 128 :], 0.0)
        for s in range(DT):
            for c in range(4):  # lk column chunks of 512
                pk = pA.tile([128, 512], F32, name=f"pk{s}_{c}", tag="pA")
                for j in range(0, DT, 2):
                    nc.tensor.matmul(
                        pk[:],
                        wk8v[:, j : j + 2, s * 128 : (s + 1) * 128],
                        enct8v[:, j : j + 2, c * 512 : (c + 1) * 512],
                        start=(j == 0),
                        stop=(j == DT - 2),
                        perf_mode=DoubleRow,
                    )
                # k = 2^-9 psum. ACT cols (lk < 1536): KT8 = fp8(k*2^3).
                # Hack cols (lk >= 1536): KT8 = fp8(k * 23.083 * 2^-3).
                scale = 2.0**-6 if c < 3 else HACK_SCALE * (2.0**-12)
                nc.vector.tensor_scalar(
                    kt8[s][:, c * 512 : (c + 1) * 512], pk[:], scale, None, Alu.mult
                )

        # ---- V projection (fp8 DR), scattered per head ----
        # V8: ACT chunks [128, H, 6, 2, 65] fp8 (x2^4; col 64 = 16.0)
        # Vb: hack tiles [128, H, 4, 65] bf16 (x2^4; col 64 = 16.0)
        # 68-wide slots keep every (h, chunk) slice 4-byte aligned; col 64 is
        # the 16.0 "ones" column (softmax denominator), cols 65-67 are padding.
        v8 = vpool.tile([128, H, ACT_CHUNKS, 2, 68], F8, name="v8", tag="v8")
        vb = vbpool.tile([128, H, 4, 68], BF16, name="vb", tag="vb")
        v8f = v8[:].rearrange("p a b c d -> p (a b c) d")
        vbf = vb[:].rearrange("p a b c -> p (a b) c")
        nc.gpsimd.memset(v8f[:, :, 64:65], 16.0)
        nc.gpsimd.memset(vbf[:, :, 64:65], 16.0)
        nc.gpsimd.memset(v8f[:, :, 65:68], 0.0)
        nc.gpsimd.memset(vbf[:, :, 65:68], 0.0)
        for t in range(LT):
            pv = pA.tile([128, D], F32, name=f"pv{t}", tag="pA")
            for j in range(0, DT, 2):
                nc.tensor.matmul(
                    pv[:],
                    enct8v[:, j : j + 2, t * 128 : (t + 1) * 128],
                    wv8v[:, j : j + 2, :],
                    start=(j == 0),
                    stop=(j == DT - 2),
                    perf_mode=DoubleRow,
                )
            pvh = pv[:].rearrange("p (h d) -> p h d", h=H)
            if t < 2 * ACT_CHUNKS:
                out = v8[:, :, t // 2, t % 2, 0:64]
            else:
                out = vb[:, :, t - 2 * ACT_CHUNKS, 0:64]
            nc.vector.tensor_scalar(out, pvh, 2.0**-5, None, Alu.mult)

        def bail():
            for rt_ in range(RT):
                yt = ypool.tile([128, D], F32, name=f"yb{rt_}", tag="y")
                nc.gpsimd.memset(yt[:], 0.0)
                nc.sync.dma_start(y_d[:, rt_ * D : (rt_ + 1) * D], yt[:])
            nc.compile.__self__ if False else None

        if PHASE <= 2:
            bail()
        # ---- attention ----
        o1 = o1pool.tile([128, RT * D], F32, name="o1", tag="o1")
        o1v = o1[:].rearrange("p (r d) -> p r d", d=D)
        e8s = [
            e8pool.tile([128, ACT_CHUNKS * 1024], F8, name=f"e8_{i}", tag="e8")
            for i in range(2)
        ]
        ebbs = [
            ebpool.tile([128, 4 * 512], BF16, name=f"ebb{i}", tag="ebb")
            for i in range(2)
        ]
        for h in range(H if PHASE >= 3 else 0):
            pr, off = h // 2, 64 * (h % 2)
            e8 = e8s[h % 2]
            ebb = ebbs[h % 2]
            e8v = e8[:].rearrange("p (t q) -> p t q", q=512)
            ktv = kt8[pr][:].rearrange("p (n w) -> p n w", w=128)
            acc = pB.tile([128, RT, 68], F32, name=f"acc{h}", tag="pB")
            for c in range(NCH):
                sc = pA.tile([128, 1024], F32, name=f"sc{h}_{c}", tag="pA")
                for tt in range(2):
                    t = 2 * c + tt
                    nc.tensor.matmul(
                        sc[:, tt * 512 : (tt + 1) * 512],
                        ktv[off : off + 64, t : LT + 1 : LT - t, :],
                        qt8v[off : off + 64, pr : DT + 1 : DT - pr, :],
                        start=True,
                        stop=True,
                        perf_mode=DoubleRow,
                        tile_position=(off, 0),
                    )
                if KATT < 2:
                    continue
                if c < ACT_CHUNKS:
                    # e8 = fp8(exp(qk/8 - EOFF)); sc psum = 2^6 qk
                    nc.scalar.activation(
                        e8[:, c * 1024 : (c + 1) * 1024],
                        sc[:],
                        mybir.ActivationFunctionType.Exp,
                        bias=moff_col[:, 0:1],
                        scale=2.0**-9,
                    )
                else:
                    # sc psum = 23.083 qk here; bits = psum + HACK_BIAS
                    nc.vector.tensor_scalar(
                        ebb[:, (c - ACT_CHUNKS) * 1024 : (c - ACT_CHUNKS + 1) * 1024]
                        .bitcast(I16),
                        sc[:],
                        HACK_BIAS,
                        None,
                        Alu.add,
                    )
                for qt_ in range(RT if KATT >= 3 else 0):
                    if c < ACT_CHUNKS:
                        nc.tensor.matmul(
                            acc[:, qt_, :],
                            e8v[:, 2 * c : 2 * c + 2, qt_ * 128 : (qt_ + 1) * 128],
                            v8[:, h, c, :, :],
                            start=(c == 0 and qt_ == 0),
                            stop=False,
                            perf_mode=DoubleRow,
                        )
                    else:
                        for tt in range(2):
                            tloc = (c - ACT_CHUNKS) * 2 + tt
                            nc.tensor.matmul(
                                acc[:, qt_, :],
                                ebb[:, tloc * 512 + qt_ * 128 :][:, :128],
                                vb[:, h, tloc, :],
                                start=False,
                                stop=(c == NCH - 1 and tt == 1 and qt_ == RT - 1),
                            )
            if KATT < 4:
                continue
            rec = spool.tile([128, RT], F32, name=f"rec{h}", tag="stat")
            nc.vector.reciprocal(rec[:], acc[:, :, 64:65])
            for qt_ in range(RT):
                # o1[qt, head cols] = num * rec + x
                nc.vector.scalar_tensor_tensor(
                    o1v[:, qt_, h * 64 : (h + 1) * 64],
                    acc[:, qt_, 0:64],
                    rec[:, qt_ : qt_ + 1],
                    xbv[:, qt_, h * 64 : (h + 1) * 64],
                    Alu.mult,
                    Alu.add,
                )

        # ---- LN1 -> out1 (bf16) ----
        out1b = ob1pool.tile([128, RT * D], BF16, name="out1b", tag="out1b")
        ob1v = out1b[:].rearrange("p (r d) -> p r d", d=D)

        def layer_norm(dst, src, name, gain_bc=None, bias_bc=None):
            bn6 = spool.tile([128, 6], F32, name=f"bn6{name}", tag="stat")
            nc.vector.bn_stats(bn6[:], src)
            mv = spool.tile([128, 2], F32, name=f"mv{name}", tag="stat")
            nc.vector.bn_aggr(mv[:], bn6[:])
            std = spool.tile([128, 1], F32, name=f"std{name}", tag="stat")
            nc.scalar.activation(
                std[:], mv[:, 1:2], mybir.ActivationFunctionType.Sqrt,
                bias=eps_col[:, 0:1],
            )
            rstd = spool.tile([128, 1], F32, name=f"rstd{name}", tag="stat")
            nc.vector.reciprocal(rstd[:], std[:])
            nc.gpsimd.tensor_scalar(
                dst, src, mv[:, 0:1], rstd[:, 0:1], Alu.subtract, Alu.mult
            )
            if gain_bc is not None:
                nc.gpsimd.tensor_tensor(dst, dst, gain_bc[:], Alu.mult)
                nc.gpsimd.tensor_tensor(dst, dst, bias_bc[:], Alu.add)

        if PHASE <= 3:
            bail()
        for qt_ in range(RT if PHASE >= 4 else 0):
            layer_norm(ob1v[:, qt_, :], o1v[:, qt_, :], f"ln1_{qt_}")

        # ---- out1^T (PE transpose via bf16 psum) ----
        o1t = o1tpool.tile([128, DT * ROWS], BF16, name="o1t", tag="o1t")
        if PHASE < 4:
            DT_T = 0
        else:
            DT_T = DT
        o1tv = o1t[:].rearrange("p (n w) -> p n w", w=ROWS)
        for dt_ in range(DT_T):
            pt = pA.tile([128, ROWS], BF16, name=f"po1t{dt_}", tag="pA")
            for rt_ in range(RT):
                nc.tensor.matmul(
                    pt[:, rt_ * 128 : (rt_ + 1) * 128],
                    ob1v[:, rt_, dt_ * 128 : (dt_ + 1) * 128],
                    identb[:],
                    is_transpose=True,
                    start=(rt_ == 0),
                    stop=(rt_ == RT - 1),
                )
            nc.vector.tensor_copy(o1tv[:, dt_, :], pt[:])

        # ---- FFN (bf16) with fused FFN2 accumulation chains ----
        h1t = h1pool.tile([128, FT * ROWS], BF16, name="h1t", tag="h1t")
        h1v = h1t[:].rearrange("p (n w) -> p n w", w=ROWS)
        w1v = w1b[:].rearrange("p (n w) -> p n w", w=DFF)
        w2v = w2b[:].rearrange("p (n w) -> p n w", w=D)
        pffs = [
            (pB if rt_ < 2 else pC).tile([128, D], F32, name=f"pff{rt_}", tag="pB" if rt_ < 2 else "pC")
            for rt_ in range(RT if PHASE >= 5 else 0)
        ]
        if PHASE <= 4:
            bail()
        for s in range(FT if PHASE >= 5 else 0):
            ph = pA.tile([128, ROWS], F32, name=f"ph{s}", tag="pA")
            for kt_ in range(DT):
                nc.tensor.matmul(
                    ph[:],
                    w1v[:, kt_, s * 128 : (s + 1) * 128],
                    o1tv[:, kt_, :],
                    start=(kt_ == 0),
                    stop=(kt_ == DT - 1),
                )
            nc.scalar.activation(
                h1v[:, s, :], ph[:], mybir.ActivationFunctionType.Relu,
                bias=b1c[:, s : s + 1],
            )
            for rt_ in range(RT):
                nc.tensor.matmul(
                    pffs[rt_][:],
                    h1v[:, s, rt_ * 128 : (rt_ + 1) * 128],
                    w2v[:, s, :],
                    start=(s == 0),
                    stop=(s == FT - 1),
                )

        # ---- y tail: +out1 residual, LN2, optional g2/b2, store ----
        g2bc = be2bc = b2bc = None
        if apply_g2b2 or add_b2:
            def bcast(name, dram):
                row = cpool.tile([1, D], F32, name=f"{name}row")
                nc.sync.dma_start(row[:], dram[None, :])
                full = cpool.tile([128, D], F32, name=f"{name}bc")
                nc.gpsimd.partition_broadcast(full[:], row[:])
                return full

            g2bc = bcast("g2", g2_d)
            be2bc = bcast("be2", be2_d)
            b2bc = bcast("b2", b2_d)

        for rt_ in range(RT if PHASE >= 5 else 0):
            yt = ypool.tile([128, D], F32, name=f"y{rt_}", tag="y")
            nc.vector.tensor_tensor(yt[:], pffs[rt_][:], ob1v[:, rt_, :], Alu.add)
            if add_b2:
                nc.vector.tensor_tensor(yt[:], yt[:], b2bc[:], Alu.add)
            layer_norm(
                yt[:], yt[:], f"ln2_{rt_}",
                gain_bc=g2bc if apply_g2b2 else None,
                bias_bc=be2bc if apply_g2b2 else None,
            )
            nc.sync.dma_start(y_d[:, rt_ * D : (rt_ + 1) * D], yt[:])

    nc.compile()
    return nc


_CACHED = {}


def _get_nc(apply_g2b2: bool = False, add_b2: bool = False):
    key = (apply_g2b2, add_b2)
    if key not in _CACHED:
        _CACHED[key] = build_program(*key)
    return _CACHED[key]


def _f8(x, scale_pow):
    return (np.asarray(x, np.float32) * (2.0**scale_pow)).astype(F8NP)


def _ktile_rows(a):
    """[K, M] -> [128, (K//128)*M]: out[p, j*M + m] = a[j*128 + p, m]."""
    K, M = a.shape
    return np.ascontiguousarray(
        a.reshape(K // 128, 128, M).transpose(1, 0, 2).reshape(128, -1)
    )


def kernel(**inputs) -> np.ndarray:
    x = np.asarray(inputs["inputs"], dtype=np.float32)
    enc = np.asarray(inputs["encoder_x"], dtype=np.float32)
    assert x.shape == (B, LQ, D) and enc.shape == (B, LK, D)
    assert int(np.asarray(inputs["n_heads"])) == H

    Wq = np.asarray(inputs["Wq"], np.float32)
    Wk = np.asarray(inputs["Wk"], np.float32)
    Wv = np.asarray(inputs["Wv"], np.float32)
    g1 = np.asarray(inputs["ln1_g"], np.float64)
    be1 = np.asarray(inputs["ln1_b"], np.float64)
    w1_raw = np.asarray(inputs["W1"], np.float64)
    w1_eff = (g1[:, None] * w1_raw).astype(np.float32)
    b1_eff = (np.asarray(inputs["b1"], np.float64) + be1 @ w1_raw).astype(np.float32)
    W2 = np.asarray(inputs["W2"], np.float32)
    b2 = np.asarray(inputs["b2"], np.float32)
    g2 = np.asarray(inputs["ln2_g"], np.float32)
    be2 = np.asarray(inputs["ln2_b"], np.float32)

    apply_g2b2 = not (np.allclose(g2, 1.0) and np.allclose(be2, 0.0))
    add_b2 = not np.allclose(b2, 0.0)
    nc = _get_nc(apply_g2b2, add_b2)

    shared = {
        "wq8": _ktile_rows(_f8(Wq, 5)),
        "wk8": _ktile_rows(_f8(Wk, 5)),
        "wv8": _ktile_rows(_f8(Wv, 5)),
        "w1b": _ktile_rows(w1_eff.astype(BF16NP)),
        "w2b": _ktile_rows(W2.astype(BF16NP)),
        "b1c": np.ascontiguousarray(_ktile_rows(b1_eff[:, None]).astype(np.float32)),
        "g2": np.ascontiguousarray(g2),
        "be2": np.ascontiguousarray(be2),
        "b2": np.ascontiguousarray(b2),
    }
    xf = x.reshape(B * LQ, D)
    in_maps = []
    for c in range(N_CORES):
        b = c // (N_CORES // B)
        xs = xf[c * ROWS : (c + 1) * ROWS]
        m = dict(shared)
        m["xt8"] = _ktile_rows(_f8(np.ascontiguousarray(xs.T), 4))
        m["xb"] = _ktile_rows(xs.astype(BF16NP))
        m["enct8"] = _ktile_rows(_f8(np.ascontiguousarray(enc[b].T), 4))
        in_maps.append(m)

    res = run_bass_kernel_spmd(nc, in_maps, core_ids=list(range(N_CORES)))
    out = np.empty((B * LQ, D), np.float32)
    for c in range(N_CORES):
        yc = res.results[c]["y"].reshape(128, RT, D).transpose(1, 0, 2).reshape(ROWS, D)
        out[c * ROWS : (c + 1) * ROWS] = yc
    return out.reshape(B, LQ, D)
